# revision 14
# baseline (speedup 1.0000x reference)
"""Trainium2 Bass kernel for the dense real-space long-range kernel
(N=6144 atoms, B=8 periodic cells, screened-Coulomb pair energy with
minimum-image convention, row-summed per atom).

batch is sorted and cross-graph pairs are masked, so the N x N problem is
block-diagonal over 8 graphs: one graph per NeuronCore.  Within a core the
pair matrix is symmetric, so only the upper-triangular macro strips are
computed: macro row m (126 atoms) covers columns [126m, cols).

Per 42-atom group (3 coord rows per atom = 126 partitions):
  f   = frac_b - frac_a          DVE tensor_scalar (broadcast col)
  r   = (f + M) - M  = round(f)  Pool tensor_scalar (magic number)
  y   = C^T(f - r)               2 accumulating fp32r matmuls (block-diag C)
  sq  = y*y                      ACT Square (2 of 3 groups) / DVE (1 of 3)
  q   = sum_k y_k^2              ones-blockdiag matmul, PSUM accum
Per macro strip:
  rt  = sqrt(q + soft^2)         ACT Sqrt (same act table set as Square ->
                                 no table reloads in the main loop)
  rcp = 1/rt                     DVE reciprocal_approx_fast
then a second batched phase (one Exp table load total):
  et  = exp(-sigma*rt)           ACT Exp
  kern = et * rcp                Pool tensor_tensor
  colacc[j] += sum_a s_a kern    fp32r matvec, PSUM accum over strips
  racc[a]    = sum_{j>diag} kern*s_j   DVE tensor_tensor_reduce
Host: E = 0.5*s*(colacc + racc) - 0.5*s^2*exp(-sigma*soft)/soft.
"""
import os
import numpy as np

DBG = set(os.environ.get("KDBG", "").split(","))

GA = 42            # atoms per row group
ROWS = 3 * GA      # 126 partitions per group tile
GPM = 3            # groups per macro
MACRO = GA * GPM   # 126 atoms per macro strip
MAGIC = 12582912.0  # 1.5 * 2**23: (x + MAGIC) - MAGIC == round(x) for |x| < 2**22
NCORES = 8
CHUNK = 512        # PSUM bank / fp32 matmul free-dim limit

_cache = {}


def _build(n_macros, cols, sigma, soft):
    import concourse.bacc as bacc
    import concourse.mybir as mybir
    import concourse.tile as tile

    f32 = mybir.dt.float32
    f32r = mybir.dt.float32r
    alu = mybir.AluOpType
    act = mybir.ActivationFunctionType

    n_groups = GPM * n_macros
    pw = -(-cols // CHUNK) * CHUNK
    soft2 = float(np.float32(soft) * np.float32(soft))

    nc = bacc.Bacc("TRN2", target_bir_lowering=False, debug=False)
    # const AP for the Sqrt bias (soft^2), registered like the built-ins
    t = nc.alloc_sbuf_tensor("const-soft2", [128, 1], f32)
    nc.gpsimd.memset(t.ap(), soft2)
    nc.const_aps.aps[(f32, soft2)] = t.ap()
    nc.all_engine_barrier()

    FB = nc.declare_dram_parameter("FB", [ROWS, cols], f32, isOutput=False)
    NEGFA = nc.declare_dram_parameter("NEGFA", [ROWS, n_groups], f32, isOutput=False)
    CB = nc.declare_dram_parameter("CB", [ROWS, ROWS], f32r, isOutput=False)
    CBN = nc.declare_dram_parameter("CBN", [ROWS, ROWS], f32r, isOutput=False)
    ONESB = nc.declare_dram_parameter("ONESB", [ROWS, GPM * MACRO], f32r, isOutput=False)
    SRCST = nc.declare_dram_parameter("SRCST", [MACRO, n_macros + 1], f32r, isOutput=False)
    SRB = nc.declare_dram_parameter("SRB", [MACRO, cols], f32, isOutput=False)
    OUT1 = nc.declare_dram_parameter("OUT1", [1, cols], f32, isOutput=True)
    OUT2 = nc.declare_dram_parameter("OUT2", [MACRO, n_macros], f32, isOutput=True)

    with tile.TileContext(nc) as tc:
        with tc.tile_pool(name="const", bufs=1) as cpool, \
             tc.tile_pool(name="work", bufs=3) as pool, \
             tc.tile_pool(name="ypsum", bufs=2, space="PSUM") as ypool, \
             tc.tile_pool(name="qpsum", bufs=1, space="PSUM") as qpool, \
             tc.tile_pool(name="apsum", bufs=1, space="PSUM") as apool:
            fb = cpool.tile([ROWS, cols], f32)
            negfa = cpool.tile([ROWS, n_groups], f32)
            cb = cpool.tile([ROWS, ROWS], f32r)
            cbn = cpool.tile([ROWS, ROWS], f32r)
            onesb = cpool.tile([ROWS, GPM * MACRO], f32r)
            srcst = cpool.tile([MACRO, n_macros + 1], f32r)
            srb = cpool.tile([MACRO, cols], f32)
            zmov = cpool.tile([ROWS, CHUNK], f32)
            zcol = cpool.tile([MACRO, 1], f32)
            raccs = []
            for m in range(n_macros - 1):
                raccs.append(cpool.tile([MACRO, 1], f32, name=f"racc{m}"))
            nc.sync.dma_start(fb[:], FB[:])
            nc.sync.dma_start(negfa[:], NEGFA[:])
            nc.sync.dma_start(cb[:], CB[:])
            nc.sync.dma_start(cbn[:], CBN[:])
            nc.sync.dma_start(onesb[:], ONESB[:])
            nc.sync.dma_start(srcst[:], SRCST[:])
            nc.sync.dma_start(srb[:], SRB[:])
            nc.gpsimd.memset(zmov[:], 0.0)
            nc.gpsimd.memset(zcol[:], 0.0)

            acc = apool.tile([1, pw], f32)
            rts = []
            for m in range(n_macros):
                w = cols - MACRO * m
                rts.append(cpool.tile([MACRO, w], f32, name=f"rt{m}"))

            # phase 1: q / rt / rcp per strip (ACT uses Square+Sqrt: one set)
            for m in range(n_macros):
                w = cols - MACRO * m
                c0 = MACRO * m
                chunks = [(c, min(w, c + CHUNK)) for c in range(0, w, CHUNK)]
                q = qpool.tile([MACRO, pw], f32, tag="q")
                for tg in range(GPM):
                    g = GPM * m + tg
                    f = pool.tile([ROWS, cols], f32r, tag="f")
                    nc.vector.tensor_scalar(f[:, 0:w], fb[:, c0:cols],
                                            negfa[:, g:g + 1], None, alu.add)
                    r = pool.tile([ROWS, cols], f32r, tag="r")
                    nc.vector.tensor_scalar(r[:, 0:w], f[:, 0:w], MAGIC, MAGIC,
                                            alu.add, alu.subtract)
                    y = ypool.tile([ROWS, pw], f32, tag="y")
                    for (a, b) in chunks:
                        nc.tensor.matmul(y[:, a:b], cb[:], f[:, a:b],
                                         start=True, stop=False)
                        nc.tensor.matmul(y[:, a:b], cbn[:], r[:, a:b],
                                         start=False, stop=True)
                    sq = pool.tile([ROWS, cols], f32r, tag="sq")
                    nc.scalar.activation(sq[:, 0:w], y[:, 0:w], act.Square)
                    ob = onesb[:, MACRO * tg:MACRO * (tg + 1)]
                    for (a, b) in chunks:
                        nc.tensor.matmul(q[:, a:b], ob, sq[:, a:b],
                                         start=(tg == 0), stop=(tg == GPM - 1))
                nc.scalar.activation(rts[m][:], q[:, 0:w], act.Sqrt, bias=soft2)

            # phase 2a: all Ln passes (one table load), u = rt + ln(rt)/sigma
            inv_sigma = float(1.0 / np.float32(sigma))
            us = []
            for m in range(n_macros):
                w = cols - MACRO * m
                us.append(cpool.tile([MACRO, w], f32, name=f"u{m}"))
            for m in range(n_macros):
                w = cols - MACRO * m
                lt = pool.tile([MACRO, cols], f32, tag="lt")
                nc.scalar.activation(lt[:, 0:w], rts[m][:], act.Ln)
                nc.vector.scalar_tensor_tensor(us[m][:], lt[:, 0:w], inv_sigma,
                                               rts[m][:], alu.mult, alu.add)
            # phase 2b: all Exp passes (one table load) + matvec + row-sums
            for m in range(n_macros):
                w = cols - MACRO * m
                c0 = MACRO * m
                kern = pool.tile([MACRO, cols], f32r, tag="kern")
                nc.scalar.activation(kern[:, 0:w], us[m][:], act.Exp, scale=-sigma)
                for (a, b) in [(c, min(w, c + CHUNK)) for c in range(0, w, CHUNK)]:
                    nc.tensor.matmul(acc[0:1, c0 + a:c0 + b], srcst[:, m:m + 1],
                                     kern[:, a:b], start=(m == 0), stop=False,
                                     skip_group_check=True)
                if w > MACRO and "nottr" not in DBG:
                    scr = pool.tile([MACRO, cols], f32, tag="scr")
                    nc.vector.scalar_tensor_tensor(
                        scr[:, 0:w - MACRO], kern[:, MACRO:w].bitcast(f32), 1.0,
                        srb[:, c0 + MACRO:cols], alu.mult, alu.mult,
                        accum_out=raccs[m][:])
            # close the PSUM accumulation group over acc with a zero matvec
            if "nocloser" not in DBG:
                for (a, b) in [(c, min(cols, c + CHUNK)) for c in range(0, cols, CHUNK)]:
                    nc.tensor.matmul(acc[0:1, a:b], srcst[:, n_macros:n_macros + 1],
                                     zmov[:, 0:b - a].bitcast(f32r), start=False, stop=True,
                                     skip_group_check=True)
            colacc = cpool.tile([1, cols], f32)
            nc.scalar.activation(colacc[:], acc[0:1, 0:cols], act.Copy)
            nc.sync.dma_start(OUT1[:], colacc[:])
            for m in range(n_macros):
                src_col = raccs[m] if m < n_macros - 1 else zcol
                nc.sync.dma_start(OUT2[:, m:m + 1], src_col[:])
    nc.compile()
    return nc


def _get_program(n_macros, cols, sigma, soft):
    key = (n_macros, cols, round(sigma, 9), round(soft, 9))
    if key not in _cache:
        _cache[key] = _build(n_macros, cols, sigma, soft)
    return _cache[key]


LAST_EXEC_TIME_NS = None


def kernel(pos, batch, cell, source, screening, softening, *, _trace=False):
    global LAST_EXEC_TIME_NS
    from concourse.bass_utils import run_bass_kernel_spmd

    pos = np.asarray(pos)
    batch = np.asarray(batch)
    cell = np.asarray(cell)
    source = np.asarray(source, dtype=np.float32)
    sigma = float(np.asarray(screening, dtype=np.float32))
    soft = float(np.asarray(softening, dtype=np.float32))

    n = pos.shape[0]
    nb = cell.shape[0]
    bi = batch.astype(np.int64)
    counts = np.bincount(bi, minlength=nb)
    starts = np.concatenate([[0], np.cumsum(counts)])
    assert nb == NCORES and np.all(np.diff(bi) >= 0)

    # host precompute in float64
    inv = np.linalg.inv(cell.astype(np.float64))
    frac = np.empty((n, 3), dtype=np.float64)
    for g in range(nb):
        i0, i1 = starts[g], starts[g + 1]
        frac[i0:i1] = pos[i0:i1].astype(np.float64) @ inv[g]
    frac32 = frac.astype(np.float32)

    namax = int(counts.max())
    n_macros = -(-namax // MACRO)
    cols = MACRO * n_macros       # padded atom count per core
    n_groups = GPM * n_macros
    diag_c = float(np.exp(-np.float64(sigma) * np.float64(soft)) / np.float64(soft))

    idx_atom = np.arange(ROWS) // 3
    idx_k = np.arange(ROWS) % 3

    in_maps = []
    for g in range(nb):
        i0, i1 = starts[g], starts[g + 1]
        ng = i1 - i0
        fpad = np.zeros((cols, 3), dtype=np.float32)
        fpad[:ng] = frac32[i0:i1]
        spad = np.zeros(cols, dtype=np.float32)
        spad[:ng] = source[i0:i1]

        fb = np.ascontiguousarray(np.tile(fpad.T, (GA, 1)))  # [126, cols]
        negfa = np.zeros((ROWS, n_groups), dtype=np.float32)
        for t in range(n_groups):
            a = t * GA + idx_atom
            negfa[:, t] = -fpad[a, idx_k]
        C = cell[g].astype(np.float32)
        cbm = np.zeros((ROWS, ROWS), dtype=np.float32)
        for i in range(GA):
            cbm[3 * i:3 * i + 3, 3 * i:3 * i + 3] = C
        onesb = np.zeros((ROWS, GPM, MACRO), dtype=np.float32)
        for t in range(GPM):
            for i in range(GA):
                onesb[3 * i:3 * i + 3, t, GA * t + i] = 1.0
        onesb = np.ascontiguousarray(onesb.reshape(ROWS, GPM * MACRO))
        srcst = np.zeros((MACRO, n_macros + 1), dtype=np.float32)
        for m in range(n_macros):
            srcst[:, m] = spad[m * MACRO: m * MACRO + MACRO]
        srbm = np.ascontiguousarray(np.broadcast_to(spad[None, :], (MACRO, cols)))
        in_maps.append({
            "FB": fb, "NEGFA": negfa, "CB": cbm, "CBN": -cbm,
            "ONESB": onesb, "SRCST": srcst, "SRB": srbm,
        })

    nc = _get_program(n_macros, cols, sigma, soft)
    res = run_bass_kernel_spmd(nc, in_maps, list(range(NCORES)), trace=_trace)
    LAST_EXEC_TIME_NS = res.exec_time_ns

    out = np.zeros((n, 1), dtype=np.float32)
    for g in range(nb):
        i0, i1 = starts[g], starts[g + 1]
        ng = i1 - i0
        colacc = res.results[g]["OUT1"][0]          # [cols]
        raccf = res.results[g]["OUT2"].T.ravel()    # [cols], atom p = 126m+i
        s = np.zeros(cols, dtype=np.float64)
        s[:ng] = source[i0:i1].astype(np.float64)
        e = 0.5 * s * (colacc.astype(np.float64) + raccf.astype(np.float64)) \
            - 0.5 * s * s * diag_c
        out[i0:i1, 0] = e[:ng].astype(np.float32)
    return out
